# revision 4
# baseline (speedup 1.0000x reference)
"""Trainium2 Bass kernel for ChannelAttention (B=16, C=512, H=W=64).

Math (per batch b):
    xf = x[b] reshaped [C, N], N = H*W = 4096
    q = Wq @ xf + bq            [64, N]
    k = Wk @ xf + bk            [64, N]
    v = Wv @ xf + bv            [64, N]
    energy = q @ k.T            [64, 64]   (contraction over N)
    attn = softmax(energy, -1)
    z = attn @ v                [64, N]
    out = Wo @ z + bo           [C, N]

Sharding: data-parallel over batch, 2 batches per core on 8 cores, no
collectives.  Each core receives its x shard plus the (host-pre-transposed)
weights and returns its out shard.

On-chip dataflow per batch (8 n-panels of 512):
  - q,k projected together in native [128(q|k), n] layout (PSUM accumulate
    over 4 c-chunks), bias added during the PSUM->SBUF copy on the Scalar
    engine (per-partition bias).
  - q|k panel transposed 128x128-wise on the PE (is_transpose matmul with an
    identity) into [n, q|k] layout; energy accumulated in PSUM over all 32
    n-chunks as qT.T @ kT.
  - v projected in native [64, n] layout, kept in SBUF for the whole batch.
  - softmax: DVE row-max (negated), ACT exp with bias=-max and accum_out
    row-sum; 1/rowsum folded into the z PSUM->SBUF copy scale.
  - attn transposed once (64x64) on the PE; z = attn @ v; out = Wo @ z + bo.

The energy path (q,k projection + energy matmuls) runs in full fp32
(4 cycles/row on the PE) because softmax is sensitive to energy error;
the v / attn@v / Wo matmuls run as float32r (1 cycle/row at free dim 512)
whose ~1e-3 relative error is benign for the output.
"""

import os

import numpy as np

# Problem shape (hardcoded; kernel.py must be self-contained).
B, C, H, W = 16, 512, 64, 64
N = H * W  # 4096
C8 = 64
P = 128
NCORES = 8
BPC = B // NCORES  # batches per core
CCH = C // P  # 4 c-chunks of 128
NP = 512  # n-panel width
NPANELS = N // NP  # 8
NSUB = NP // P  # 4 transpose subtiles per panel

# Matmul dtype knobs ("f32" = exact, "f32r" = fast single-pass).
QK_DT = os.environ.get("CHATT_QK_DT", "f32")
V_DT = os.environ.get("CHATT_V_DT", "f32r")
ZO_DT = os.environ.get("CHATT_ZO_DT", "f32r")

_CACHE = {}
LAST_RESULTS = None


def _build_program():
    import concourse.bass as bass  # noqa: F401
    import concourse.mybir as mybir
    import concourse.tile as tile
    from concourse import bacc
    from concourse.masks import make_identity

    f32 = mybir.dt.float32
    f32r = mybir.dt.float32r

    def mm_cast(ap, kind):
        return ap.bitcast(f32r) if kind == "f32r" else ap

    nc = bacc.Bacc("TRN2", target_bir_lowering=False)

    x_h = nc.dram_tensor("x", [BPC, C, N], f32, kind="ExternalInput")
    wqk_h = nc.dram_tensor("w_qkt", [C, P], f32, kind="ExternalInput")
    wv_h = nc.dram_tensor("w_vt", [C, C8], f32, kind="ExternalInput")
    wo_h = nc.dram_tensor("w_ot", [C8, C], f32, kind="ExternalInput")
    bqk_h = nc.dram_tensor("b_qk", [P], f32, kind="ExternalInput")
    bv_h = nc.dram_tensor("b_v", [C8], f32, kind="ExternalInput")
    bo_h = nc.dram_tensor("b_o", [C], f32, kind="ExternalInput")
    y_h = nc.dram_tensor("y", [BPC, C, N], f32, kind="ExternalOutput")

    x_ap = x_h.ap()
    y_ap = y_h.ap()

    from contextlib import ExitStack

    with tile.TileContext(nc) as tc, ExitStack() as ctx:
        consts = ctx.enter_context(tc.tile_pool(name="consts", bufs=1))
        xp = ctx.enter_context(tc.tile_pool(name="xp", bufs=3))
        qkp = ctx.enter_context(tc.tile_pool(name="qkp", bufs=2))
        qktp = ctx.enter_context(tc.tile_pool(name="qktp", bufs=3))
        vp = ctx.enter_context(tc.tile_pool(name="vp", bufs=2))
        zp = ctx.enter_context(tc.tile_pool(name="zp", bufs=2))
        op = ctx.enter_context(tc.tile_pool(name="op", bufs=2))
        smallp = ctx.enter_context(tc.tile_pool(name="smallp", bufs=4))
        # PSUM: 8 banks total -> proj 3 + transpose 2 + energy 1 + z/out 2
        pp = ctx.enter_context(tc.tile_pool(name="pp", bufs=3, space="PSUM"))
        pt = ctx.enter_context(tc.tile_pool(name="pt", bufs=2, space="PSUM"))
        pe = ctx.enter_context(tc.tile_pool(name="pe", bufs=1, space="PSUM"))
        pzo = ctx.enter_context(tc.tile_pool(name="pzo", bufs=2, space="PSUM"))

        # One-time constants.
        wqk_sb = consts.tile([P, CCH, P], f32)
        nc.sync.dma_start(wqk_sb, wqk_h.ap().rearrange("(co ci) m -> ci co m", ci=P))
        wv_sb = consts.tile([P, CCH, C8], f32)
        nc.sync.dma_start(wv_sb, wv_h.ap().rearrange("(co ci) m -> ci co m", ci=P))
        wo_sb = consts.tile([C8, C], f32)
        nc.sync.dma_start(wo_sb, wo_h.ap())
        bqk_sb = consts.tile([P, 1], f32)
        nc.sync.dma_start(bqk_sb, bqk_h.ap()[:, None])
        bv_sb = consts.tile([C8, 1], f32)
        nc.sync.dma_start(bv_sb, bv_h.ap()[:, None])
        bo_sb = consts.tile([P, CCH], f32)
        nc.sync.dma_start(bo_sb, bo_h.ap().rearrange("(mo mi) -> mi mo", mi=P))
        ident = consts.tile([P, P], f32)
        make_identity(nc, ident)

        Identity = mybir.ActivationFunctionType.Identity
        Copy = mybir.ActivationFunctionType.Copy
        Exp = mybir.ActivationFunctionType.Exp

        for b in range(BPC):
            xb = x_ap[b].rearrange("(co ci) n -> ci co n", ci=P)
            yb = y_ap[b].rearrange("(mo mi) n -> mi mo n", mi=P)

            energy = pe.tile([C8, C8], f32, tag="energy", name=f"energy_{b}")
            v_sb = vp.tile([C8, N], f32, tag="v", name=f"v_{b}")

            # ---- Phase A: projections + energy accumulation ----
            for p in range(NPANELS):
                nsl = slice(p * NP, (p + 1) * NP)
                xf = xp.tile([P, CCH, NP], f32, tag="xf", name=f"xf_{b}_{p}")
                nc.sync.dma_start(xf, xb[:, :, nsl])

                qk_ps = pp.tile([P, NP], f32, tag="proj", name=f"qkps_{b}_{p}")
                for co in range(CCH):
                    nc.tensor.matmul(
                        qk_ps,
                        mm_cast(wqk_sb[:, co, :], QK_DT),
                        mm_cast(xf[:, co, :], QK_DT),
                        start=(co == 0),
                        stop=(co == CCH - 1),
                    )
                qk_sb = qkp.tile([P, NP], f32, tag="qk", name=f"qk_{b}_{p}")
                nc.scalar.activation(qk_sb, qk_ps, Identity, bias=bqk_sb, scale=1.0)

                v_ps = pp.tile([C8, NP], f32, tag="proj", name=f"vps_{b}_{p}")
                for co in range(CCH):
                    nc.tensor.matmul(
                        v_ps,
                        mm_cast(wv_sb[:, co, :], V_DT),
                        mm_cast(xf[:, co, :], V_DT),
                        start=(co == 0),
                        stop=(co == CCH - 1),
                    )
                nc.scalar.activation(
                    v_sb[:, nsl], v_ps, Identity, bias=bv_sb, scale=1.0
                )

                for ns in range(NSUB):
                    t_ps = pt.tile([P, P], f32, tag="tp", name=f"tps_{b}_{p}_{ns}")
                    nc.tensor.transpose(
                        t_ps, qk_sb[:, ns * P : (ns + 1) * P], ident
                    )
                    qkt_sb = qktp.tile([P, P], f32, tag="qkt", name=f"qkt_{b}_{p}_{ns}")
                    nc.vector.tensor_copy(qkt_sb, t_ps)
                    nc.tensor.matmul(
                        energy,
                        qkt_sb[:, 0:C8],
                        qkt_sb[:, C8:P],
                        start=(p == 0 and ns == 0),
                        stop=(p == NPANELS - 1 and ns == NSUB - 1),
                    )

            # ---- Phase B: softmax + attn@v + Wo ----
            negmax = smallp.tile([C8, 1], f32, tag="negmax", name=f"negmax_{b}")
            nc.vector.reduce_max(
                negmax, energy, axis=mybir.AxisListType.X, negate=True
            )
            attn = smallp.tile([C8, C8], f32, tag="attn", name=f"attn_{b}")
            rowsum = smallp.tile([C8, 1], f32, tag="rowsum", name=f"rowsum_{b}")
            nc.scalar.activation(
                attn, energy, Exp, bias=negmax, scale=1.0, accum_out=rowsum
            )
            recip = smallp.tile([C8, 1], f32, tag="recip", name=f"recip_{b}")
            nc.vector.reciprocal(recip, rowsum)

            at_ps = pt.tile([C8, C8], f32, tag="tp", name=f"atps_{b}")
            nc.tensor.transpose(at_ps, attn, ident[:C8, :C8])
            attnT = smallp.tile([C8, C8], f32, tag="attnT", name=f"attnT_{b}")
            nc.vector.tensor_copy(attnT, at_ps)

            for p in range(NPANELS):
                nsl = slice(p * NP, (p + 1) * NP)
                z_ps = pzo.tile([C8, NP], f32, tag="zo", name=f"zps_{b}_{p}")
                nc.tensor.matmul(
                    z_ps,
                    mm_cast(attnT, ZO_DT),
                    mm_cast(v_sb[:, nsl], ZO_DT),
                    start=True,
                    stop=True,
                )
                z_sb = zp.tile([C8, NP], f32, tag="z", name=f"z_{b}_{p}")
                nc.scalar.activation(z_sb, z_ps, Copy, scale=recip)

                o_sb = op.tile([P, CCH, NP], f32, tag="o", name=f"o_{b}_{p}")
                for mo in range(CCH):
                    o_ps = pzo.tile([P, NP], f32, tag="zo", name=f"ops_{b}_{p}_{mo}")
                    nc.tensor.matmul(
                        o_ps,
                        mm_cast(wo_sb[:, mo * P : (mo + 1) * P], ZO_DT),
                        mm_cast(z_sb, ZO_DT),
                        start=True,
                        stop=True,
                    )
                    nc.scalar.activation(
                        o_sb[:, mo, :], o_ps, Identity, bias=bo_sb[:, mo : mo + 1],
                        scale=1.0,
                    )
                nc.sync.dma_start(yb[:, :, nsl], o_sb)

    nc.compile()
    return nc


def _get_program():
    key = (QK_DT, V_DT, ZO_DT)
    if key not in _CACHE:
        _CACHE[key] = _build_program()
    return _CACHE[key]


def _host_inputs(x, Wq, bq, Wk, bk, Wv, bv, Wo, bo):
    """Build the per-core input maps (host-side shard + weight transposes)."""
    x = np.ascontiguousarray(x, dtype=np.float32).reshape(B, C, N)
    w_qkt = np.ascontiguousarray(
        np.concatenate([Wq, Wk], axis=0).T.astype(np.float32)
    )  # [C, 128]
    w_vt = np.ascontiguousarray(Wv.T.astype(np.float32))  # [C, 64]
    w_ot = np.ascontiguousarray(Wo.T.astype(np.float32))  # [64, C]
    b_qk = np.ascontiguousarray(
        np.concatenate([bq, bk], axis=0).astype(np.float32)
    )  # [128]
    b_v = np.ascontiguousarray(bv.astype(np.float32))
    b_o = np.ascontiguousarray(bo.astype(np.float32))

    in_maps = []
    for i in range(NCORES):
        in_maps.append(
            {
                "x": np.ascontiguousarray(x[i * BPC : (i + 1) * BPC]),
                "w_qkt": w_qkt,
                "w_vt": w_vt,
                "w_ot": w_ot,
                "b_qk": b_qk,
                "b_v": b_v,
                "b_o": b_o,
            }
        )
    return in_maps


def kernel(**inputs):
    global LAST_RESULTS
    from concourse.bass_utils import run_bass_kernel_spmd

    nc = _get_program()
    in_maps = _host_inputs(**inputs)
    res = run_bass_kernel_spmd(nc, in_maps, core_ids=list(range(NCORES)))
    LAST_RESULTS = res
    out = np.concatenate([r["y"] for r in res.results], axis=0)
    return out.reshape(B, C, H, W).astype(np.float32)
